# revision 1
# baseline (speedup 1.0000x reference)
"""Distributed Trainium2 kernel for nn_CAnet (vq_codebook).

Sharding: H-axis split into 8 row-bands (all 4 batch elements per core).
Halos are materialized host-side by overlapping the input row slices, so the
device graph needs no collectives. The device computes the dominant stages
(conv0 3x3, conv1 3x3 d2, 2x2 avgpool, conv2 3x3 d3, all with ReLU) as
PSUM-accumulated per-tap matmuls in bf16. The cheap global stages (CBAM
attention, soft-VQ encoding, bilinear upsample, classifier) run host-side.
"""

import numpy as np
import ml_dtypes

from concourse import bacc, mybir, tile
from concourse.bass_utils import run_bass_kernel_spmd

F32 = mybir.dt.float32
BF16 = mybir.dt.bfloat16
F32R = mybir.dt.float32r

B = 4
CIN = 103
CF = 64
H = W = 256
# per-core row geometry (stride 30 in c1/c2 rows, 15 in c3 rows)
XR = 48     # x rows per shard (30*i .. 30*i+48, zero-padded at the bottom)
C1R = 46    # c1 rows computed per shard  (global 30*i ..)
C2R = 42    # c2 rows per shard           (global 30*i ..)
PR = 21     # pooled rows per shard       (global 15*i ..)
C3R = 15    # c3 rows per shard           (global 15*i ..)
C1W, C2W, PW, C3W = 254, 250, 125, 119

_CACHE = {}
LAST_RESULT = None


def _build(nc):
    x_d = nc.dram_tensor("x", [B, CIN, XR, W], BF16, kind="ExternalInput").ap()
    w0_d = nc.dram_tensor("w0t", [CIN, 9 * CF], F32, kind="ExternalInput").ap()
    w1p_d = nc.dram_tensor("w1p", [2 * CF, 3 * CF], F32, kind="ExternalInput").ap()
    w1r_d = nc.dram_tensor("w1r", [CF, 3 * CF], F32, kind="ExternalInput").ap()
    w2p_d = nc.dram_tensor("w2p", [2 * CF, 3 * CF], F32, kind="ExternalInput").ap()
    w2r_d = nc.dram_tensor("w2r", [CF, 3 * CF], F32, kind="ExternalInput").ap()
    b0_d = nc.dram_tensor("b0", [CF, 1], F32, kind="ExternalInput").ap()
    b1_d = nc.dram_tensor("b1", [CF, 1], F32, kind="ExternalInput").ap()
    b2_d = nc.dram_tensor("b2", [CF, 1], F32, kind="ExternalInput").ap()
    c1_d = nc.dram_tensor("c1", [B, CF, C1R, C1W], BF16, kind="ExternalOutput").ap()
    c2_d = nc.dram_tensor("c2", [B, CF, C2R, C2W], BF16, kind="ExternalOutput").ap()
    c3_d = nc.dram_tensor("c3", [B, CF, C3R, C3W], BF16, kind="ExternalOutput").ap()

    RELU = mybir.ActivationFunctionType.Relu

    with tile.TileContext(nc) as tc:
        with (
            tc.tile_pool(name="wpool", bufs=1) as wpool,
            tc.tile_pool(name="xb", bufs=1) as xbp,
            tc.tile_pool(name="act", bufs=2) as actp,
            tc.tile_pool(name="ps", bufs=6, space="PSUM") as psp,
        ):
            # weights: load f32, convert once to bf16
            w0f = wpool.tile([CIN, 9 * CF], F32, tag="w0f")
            w1pf = wpool.tile([2 * CF, 3 * CF], F32, tag="w1pf")
            w1rf = wpool.tile([CF, 3 * CF], F32, tag="w1rf")
            w2pf = wpool.tile([2 * CF, 3 * CF], F32, tag="w2pf")
            w2rf = wpool.tile([CF, 3 * CF], F32, tag="w2rf")
            w0b = wpool.tile([CIN, 9 * CF], BF16, tag="w0b")
            w1pb = wpool.tile([2 * CF, 3 * CF], BF16, tag="w1pb")
            w1rb = wpool.tile([CF, 3 * CF], BF16, tag="w1rb")
            w2pb = wpool.tile([2 * CF, 3 * CF], BF16, tag="w2pb")
            w2rb = wpool.tile([CF, 3 * CF], BF16, tag="w2rb")
            b0s = wpool.tile([CF, 1], F32, tag="b0s")
            b1s = wpool.tile([CF, 1], F32, tag="b1s")
            b2s = wpool.tile([CF, 1], F32, tag="b2s")
            nc.sync.dma_start(out=w0f[:, :], in_=w0_d[:, :])
            nc.sync.dma_start(out=w1pf[:, :], in_=w1p_d[:, :])
            nc.sync.dma_start(out=w1rf[:, :], in_=w1r_d[:, :])
            nc.sync.dma_start(out=w2pf[:, :], in_=w2p_d[:, :])
            nc.sync.dma_start(out=w2rf[:, :], in_=w2r_d[:, :])
            nc.sync.dma_start(out=b0s[:, :], in_=b0_d[:, :])
            nc.sync.dma_start(out=b1s[:, :], in_=b1_d[:, :])
            nc.sync.dma_start(out=b2s[:, :], in_=b2_d[:, :])
            nc.vector.tensor_copy(out=w0b[:, :], in_=w0f[:, :])
            nc.vector.tensor_copy(out=w1pb[:, :], in_=w1pf[:, :])
            nc.vector.tensor_copy(out=w1rb[:, :], in_=w1rf[:, :])
            nc.vector.tensor_copy(out=w2pb[:, :], in_=w2pf[:, :])
            nc.vector.tensor_copy(out=w2rb[:, :], in_=w2rf[:, :])

            for b in range(B):
                xb = xbp.tile([CIN, XR, W], BF16, tag="xb")
                nc.sync.dma_start(out=xb[:, :, :], in_=x_d[b, :, :, :])

                c1s = actp.tile([2 * CF, C1R, C1W], BF16, tag="c1s")
                # conv0: 3x3 valid, 103 -> 64. Two row-chunks run concurrently
                # in the two PE column halves (col-tiling via tile_position).
                for yo in range(0, C1R, 4):
                    nhalf = 2 if yo + 4 <= C1R else 1
                    ps = psp.tile([2 * CF, 2, C1W], F32, tag="ps")
                    k = 0
                    for dy in range(3):
                        for dx in range(3):
                            t = dy * 3 + dx
                            for h in range(nhalf):
                                nc.tensor.matmul(
                                    out=ps[h * CF:(h + 1) * CF, :, :],
                                    lhsT=w0b[:, t * CF:(t + 1) * CF],
                                    rhs=xb[:, yo + 2 * h + dy:yo + 2 * h + dy + 2,
                                           dx:dx + C1W],
                                    start=(k == 0), stop=(k == 8),
                                    tile_position=(0, h * CF),
                                )
                            k += 1
                    for h in range(nhalf):
                        yh = yo + 2 * h
                        nc.scalar.activation(
                            out=c1s[0:CF, yh:yh + 2, :],
                            in_=ps[h * CF:(h + 1) * CF, :, :],
                            func=RELU, bias=b0s[:, 0:1],
                        )
                        if yh >= 2:
                            nc.vector.tensor_copy(
                                out=c1s[CF:2 * CF, yh - 2:yh, :],
                                in_=c1s[0:CF, yh:yh + 2, :],
                            )
                    nc.sync.dma_start(
                        out=c1_d[b, :, yo:yo + 2 * nhalf, :],
                        in_=c1s[0:CF, yo:yo + 2 * nhalf, :]
                    )

                c2s = actp.tile([CF, C2R, C2W], BF16, tag="c2s")
                # conv1: 3x3 dilation 2, valid, 64 -> 64; dy0+dy1 packed into
                # a 128-deep contraction, two row-chunks col-tiled.
                for yo in range(0, C2R, 4):
                    nhalf = 2 if yo + 4 <= C2R else 1
                    ps = psp.tile([2 * CF, 2, C2W], F32, tag="ps")
                    for dx in range(3):
                        for h in range(nhalf):
                            nc.tensor.matmul(
                                out=ps[h * CF:(h + 1) * CF, :, :],
                                lhsT=w1pb[:, dx * CF:(dx + 1) * CF],
                                rhs=c1s[0:2 * CF, yo + 2 * h:yo + 2 * h + 2,
                                        2 * dx:2 * dx + C2W],
                                start=(dx == 0), stop=False,
                                tile_position=(0, h * CF),
                            )
                    for dx in range(3):
                        for h in range(nhalf):
                            nc.tensor.matmul(
                                out=ps[h * CF:(h + 1) * CF, :, :],
                                lhsT=w1rb[:, dx * CF:(dx + 1) * CF],
                                rhs=c1s[0:CF, yo + 2 * h + 4:yo + 2 * h + 6,
                                        2 * dx:2 * dx + C2W],
                                start=False, stop=(dx == 2),
                                tile_position=(0, h * CF),
                            )
                    for h in range(nhalf):
                        yh = yo + 2 * h
                        nc.scalar.activation(
                            out=c2s[:, yh:yh + 2, :],
                            in_=ps[h * CF:(h + 1) * CF, :, :],
                            func=RELU, bias=b1s[:, 0:1],
                        )
                    nc.sync.dma_start(
                        out=c2_d[b, :, yo:yo + 2 * nhalf, :],
                        in_=c2s[:, yo:yo + 2 * nhalf, :]
                    )

                # 2x2 average pool
                pool = actp.tile([2 * CF, PR, PW], BF16, tag="pool")
                a00 = c2s[:, 0:2 * PR:2, 0:2 * PW:2]
                a01 = c2s[:, 0:2 * PR:2, 1:2 * PW:2]
                a10 = c2s[:, 1:2 * PR:2, 0:2 * PW:2]
                a11 = c2s[:, 1:2 * PR:2, 1:2 * PW:2]
                pl = pool[0:CF, :, :]
                nc.vector.tensor_add(out=pl, in0=a00, in1=a01)
                nc.vector.tensor_add(out=pl, in0=pl, in1=a10)
                nc.vector.tensor_add(out=pl, in0=pl, in1=a11)
                nc.vector.tensor_scalar_mul(out=pl, in0=pl, scalar1=0.25)
                nc.vector.tensor_copy(
                    out=pool[CF:2 * CF, 0:PR - 3, :],
                    in_=pool[0:CF, 3:PR, :],
                )

                c3s = actp.tile([CF, C3R, C3W], BF16, tag="c3s")
                # conv2: 3x3 dilation 3, valid, 64 -> 64; dy0+dy1 packed,
                # two 4-row chunks col-tiled.
                for so in range(0, C3R, 8):
                    ns = [min(4, C3R - so), min(4, max(0, C3R - so - 4))]
                    nhalf = 2 if ns[1] > 0 else 1
                    ps = psp.tile([2 * CF, 4, C3W], F32, tag="ps")
                    for dx in range(3):
                        for h in range(nhalf):
                            nc.tensor.matmul(
                                out=ps[h * CF:(h + 1) * CF, 0:ns[h], :],
                                lhsT=w2pb[:, dx * CF:(dx + 1) * CF],
                                rhs=pool[0:2 * CF, so + 4 * h:so + 4 * h + ns[h],
                                         3 * dx:3 * dx + C3W],
                                start=(dx == 0), stop=False,
                                tile_position=(0, h * CF),
                            )
                    for dx in range(3):
                        for h in range(nhalf):
                            nc.tensor.matmul(
                                out=ps[h * CF:(h + 1) * CF, 0:ns[h], :],
                                lhsT=w2rb[:, dx * CF:(dx + 1) * CF],
                                rhs=pool[0:CF, so + 4 * h + 6:so + 4 * h + 6 + ns[h],
                                         3 * dx:3 * dx + C3W],
                                start=False, stop=(dx == 2),
                                tile_position=(0, h * CF),
                            )
                    for h in range(nhalf):
                        sh = so + 4 * h
                        nc.scalar.activation(
                            out=c3s[:, sh:sh + ns[h], :],
                            in_=ps[h * CF:(h + 1) * CF, 0:ns[h], :],
                            func=RELU, bias=b2s[:, 0:1],
                        )
                    nc.sync.dma_start(
                        out=c3_d[b, :, so:so + ns[0] + ns[1], :],
                        in_=c3s[:, so:so + ns[0] + ns[1], :]
                    )
    nc.compile()
    return nc


def _get_compiled():
    if "nc" not in _CACHE:
        _CACHE["nc"] = _build(
            bacc.Bacc("TRN2", target_bir_lowering=False, debug=False, num_devices=8)
        )
    return _CACHE["nc"]


def _upsample(x, Ho, Wo):
    """bilinear, align_corners=True, float32"""
    def mat1(n_out, n_in):
        idx = np.arange(n_out, dtype=np.float64) * ((n_in - 1) / (n_out - 1))
        lo = np.clip(np.floor(idx).astype(np.int64), 0, n_in - 2)
        f = (idx - lo).astype(np.float32)
        return lo, f
    lo, f = mat1(Ho, x.shape[2])
    x = x[:, :, lo, :] * (1.0 - f)[None, None, :, None] \
        + x[:, :, lo + 1, :] * f[None, None, :, None]
    lo, f = mat1(Wo, x.shape[3])
    x = x[:, :, :, lo] * (1.0 - f)[None, None, None, :] \
        + x[:, :, :, lo + 1] * f[None, None, None, :]
    return x.astype(np.float32)


def kernel(x, conv0_w, conv0_b, conv1_w, conv1_b, conv2_w, conv2_b, convp_w,
           fc1_w, fc2_w, enc_w, codewords, scale, attn_w, attn_b,
           bn_w, bn_b, cls_w, cls_b):
    global LAST_RESULT
    x = np.asarray(x, np.float32)
    to32 = lambda a: np.asarray(a, np.float32)
    (conv0_w, conv0_b, conv1_w, conv1_b, conv2_w, conv2_b, convp_w, fc1_w,
     fc2_w, enc_w, codewords, scale, attn_w, attn_b, bn_w, bn_b, cls_w,
     cls_b) = map(to32, (conv0_w, conv0_b, conv1_w, conv1_b, conv2_w, conv2_b,
                         convp_w, fc1_w, fc2_w, enc_w, codewords, scale,
                         attn_w, attn_b, bn_w, bn_b, cls_w, cls_b))

    nc = _get_compiled()

    w0t = np.ascontiguousarray(
        conv0_w.transpose(1, 2, 3, 0).reshape(CIN, 9 * CF))
    w1t_full = conv1_w.transpose(1, 2, 3, 0).reshape(CF, 3, 3, CF)
    w1p = np.zeros((2 * CF, 3 * CF), np.float32)
    for dx in range(3):
        w1p[:CF, dx * CF:(dx + 1) * CF] = w1t_full[:, 0, dx, :]
        w1p[CF:, dx * CF:(dx + 1) * CF] = w1t_full[:, 1, dx, :]
    w1r = np.ascontiguousarray(w1t_full[:, 2, :, :].reshape(CF, 3 * CF))
    w2t_full = conv2_w.transpose(1, 2, 3, 0).reshape(CF, 3, 3, CF)
    w2p = np.zeros((2 * CF, 3 * CF), np.float32)
    for dx in range(3):
        w2p[:CF, dx * CF:(dx + 1) * CF] = w2t_full[:, 0, dx, :]
        w2p[CF:, dx * CF:(dx + 1) * CF] = w2t_full[:, 1, dx, :]
    w2r = np.ascontiguousarray(w2t_full[:, 2, :, :].reshape(CF, 3 * CF))
    common = {
        "w0t": w0t, "w1p": w1p, "w1r": w1r, "w2p": w2p, "w2r": w2r,
        "b0": conv0_b.reshape(CF, 1).copy(),
        "b1": conv1_b.reshape(CF, 1).copy(),
        "b2": conv2_b.reshape(CF, 1).copy(),
    }
    in_maps = []
    for i in range(8):
        r0 = 30 * i
        xs = np.zeros((B, CIN, XR, W), np.float32)
        nr = min(XR, H - r0)
        xs[:, :, :nr, :] = x[:, :, r0:r0 + nr, :]
        in_maps.append({"x": xs.astype(ml_dtypes.bfloat16), **common})

    res = run_bass_kernel_spmd(nc, in_maps, core_ids=list(range(8)))
    LAST_RESULT = res

    c1 = np.empty((B, CF, 254, 254), np.float32)
    c2 = np.empty((B, CF, 250, 250), np.float32)
    c3 = np.empty((B, CF, 119, 119), np.float32)
    for i in range(8):
        r = res.results[i]
        s1 = np.asarray(r["c1"]).astype(np.float32)
        s2 = np.asarray(r["c2"]).astype(np.float32)
        s3 = np.asarray(r["c3"]).astype(np.float32)
        n1 = 30 if i < 7 else 254 - 210
        n2 = 30 if i < 7 else 250 - 210
        n3 = 15 if i < 7 else 119 - 105
        c1[:, :, 30 * i:30 * i + n1, :] = s1[:, :, :n1, :]
        c2[:, :, 30 * i:30 * i + n2, :] = s2[:, :, :n2, :]
        c3[:, :, 15 * i:15 * i + n3, :] = s3[:, :, :n3, :]

    # ---- host: CBAM channel attention ----
    sig = lambda v: 1.0 / (1.0 + np.exp(-v))

    def fc(v):
        return np.maximum(v @ fc1_w.T, 0.0) @ fc2_w.T

    gate_c = sig(fc(c3.mean((2, 3))) + fc(c3.max((2, 3))))
    xg = gate_c[:, :, None, None] * c3
    # ---- spatial attention: 7x7 conv, pad 3, 2 -> 1 channel ----
    sp = np.stack([xg.mean(1), xg.max(1)], 1)          # (B,2,119,119)
    pad = convp_w.shape[-1] // 2
    spp = np.pad(sp, ((0, 0), (0, 0), (pad, pad), (pad, pad)))
    conv_sp = np.zeros((B, C3W, C3W), np.float32)
    for c in range(2):
        for dy in range(7):
            for dx in range(7):
                conv_sp += convp_w[0, c, dy, dx] * \
                    spp[:, c, dy:dy + C3W, dx:dx + C3W]
    x_p = sig(conv_sp)[:, None, :, :] * c3
    xx = c3 + x_p
    # ---- soft-VQ context encoding (batch folded into n, as in reference) ----
    K, D = codewords.shape
    feat = np.maximum(np.tensordot(enc_w, xx, axes=([1], [1])), 0.0)
    feat = feat.transpose(1, 0, 2, 3)                  # (B, D, h, w)
    Zf = feat.reshape(1, D, -1)[0].T                   # (n, D)
    norm = ((Zf ** 2).sum(-1, keepdims=True)
            + (codewords ** 2).sum(-1)[None, :]
            - 2.0 * Zf @ codewords.T)
    logit = scale[None, :] * norm
    logit -= logit.max(1, keepdims=True)
    Aexp = np.exp(logit)
    A = Aexp / Aexp.sum(1, keepdims=True)              # (n, K)
    E = A.T @ Zf - A.sum(0)[:, None] * codewords       # (K, D)
    mu = E.mean(1, keepdims=True)
    var = ((E - mu) ** 2).mean(1, keepdims=True)
    En = (E - mu) / np.sqrt(var + 1e-5) * bn_w[:, None] + bn_b[:, None]
    E_sum = np.maximum(En, 0.0).sum(0)[None, :]        # (1, D)
    gamma = sig(E_sum @ attn_w.T + attn_b).reshape(-1, CF, 1, 1)
    xx = xx + xx * gamma
    # ---- upsample + concat + classifier ----
    cat = np.concatenate(
        [_upsample(c1, H, W), _upsample(c2, H, W), _upsample(xx, H, W)], 1)
    out = np.tensordot(cls_w, cat, axes=([1], [1])).transpose(1, 0, 2, 3)
    out = out + cls_b[None, :, None, None]
    return np.ascontiguousarray(out.astype(np.float32))



# revision 7
# speedup vs baseline: 2.0011x; 2.0011x over previous
"""Distributed Trainium2 kernel for nn_CAnet (vq_codebook).

Sharding: (batch, H-half) -> 8 cores. Core i handles batch i//2, H-half i%2.
Halos are materialized host-side by overlapping the input row slabs, so the
device graph needs no collectives. The device computes the dominant stages
(conv0 3x3, conv1 3x3 d2, 2x2 avgpool, conv2 3x3 d3, all with ReLU) as
PSUM-accumulated per-tap matmuls in bf16. The cheap global stages (CBAM
attention, soft-VQ encoding, bilinear upsample, classifier) run host-side.

The avgpool's 0.25 scale is folded into the conv2 weights host-side, so the
device pool stage is two adds (row-add contiguous, col-add strided).
"""

import numpy as np
import ml_dtypes

from concourse import bacc, mybir, tile
from concourse.bass_utils import run_bass_kernel_spmd

F32 = mybir.dt.float32
BF16 = mybir.dt.bfloat16

B = 4
CIN = 103
CF = 64
H = W = 256
# per-core geometry: one batch, one H-half (+halos, zero-padded at the edge)
XR = 138    # x rows per shard (half0: 0..138, half1: 120..256 padded to 138)
C1R = 136   # c1 rows computed per shard
C2R = 132   # c2 rows per shard
PR = 66     # pooled rows per shard
C3R = 60    # c3 rows per shard
C1W, C2W, PW, C3W = 254, 250, 125, 119

_CACHE = {}
LAST_RESULT = None


def _build(nc):
    x_d = nc.dram_tensor("x", [CIN, XR, W], BF16, kind="ExternalInput").ap()
    w0_d = nc.dram_tensor("w0t", [CIN, 9 * CF], F32, kind="ExternalInput").ap()
    w1p_d = nc.dram_tensor("w1p", [2 * CF, 3 * CF], F32, kind="ExternalInput").ap()
    w1r_d = nc.dram_tensor("w1r", [CF, 3 * CF], F32, kind="ExternalInput").ap()
    w2p_d = nc.dram_tensor("w2p", [2 * CF, 3 * CF], F32, kind="ExternalInput").ap()
    w2r_d = nc.dram_tensor("w2r", [CF, 3 * CF], F32, kind="ExternalInput").ap()
    b0_d = nc.dram_tensor("b0", [CF, 1], F32, kind="ExternalInput").ap()
    b1_d = nc.dram_tensor("b1", [CF, 1], F32, kind="ExternalInput").ap()
    b2_d = nc.dram_tensor("b2", [CF, 1], F32, kind="ExternalInput").ap()
    c1_d = nc.dram_tensor("c1", [CF, C1R, C1W], BF16, kind="ExternalOutput").ap()
    c2_d = nc.dram_tensor("c2", [CF, C2R, C2W], BF16, kind="ExternalOutput").ap()
    c3_d = nc.dram_tensor("c3", [CF, C3R, C3W], BF16, kind="ExternalOutput").ap()

    RELU = mybir.ActivationFunctionType.Relu

    with tile.TileContext(nc) as tc:
        with (
            tc.tile_pool(name="wpool", bufs=1) as wpool,
            tc.tile_pool(name="big", bufs=1) as bigp,
            tc.tile_pool(name="c1p", bufs=1) as c1p,
            tc.tile_pool(name="poolp", bufs=1) as poolp,
            tc.tile_pool(name="c3p", bufs=1) as c3p,
            tc.tile_pool(name="rs", bufs=2) as rsp,
            tc.tile_pool(name="ps", bufs=6, space="PSUM") as psp,
        ):
            # weights: load f32, convert once to bf16
            w0f = wpool.tile([CIN, 9 * CF], F32, tag="w0f")
            w1pf = wpool.tile([2 * CF, 3 * CF], F32, tag="w1pf")
            w1rf = wpool.tile([CF, 3 * CF], F32, tag="w1rf")
            w2pf = wpool.tile([2 * CF, 3 * CF], F32, tag="w2pf")
            w2rf = wpool.tile([CF, 3 * CF], F32, tag="w2rf")
            w0b = wpool.tile([CIN, 9 * CF], BF16, tag="w0b")
            w1pb = wpool.tile([2 * CF, 3 * CF], BF16, tag="w1pb")
            w1rb = wpool.tile([CF, 3 * CF], BF16, tag="w1rb")
            w2pb = wpool.tile([2 * CF, 3 * CF], BF16, tag="w2pb")
            w2rb = wpool.tile([CF, 3 * CF], BF16, tag="w2rb")
            b0s = wpool.tile([CF, 1], F32, tag="b0s")
            b1s = wpool.tile([CF, 1], F32, tag="b1s")
            b2s = wpool.tile([CF, 1], F32, tag="b2s")
            nc.sync.dma_start(out=w0f[:, :], in_=w0_d[:, :])
            nc.sync.dma_start(out=w1pf[:, :], in_=w1p_d[:, :])
            nc.sync.dma_start(out=w1rf[:, :], in_=w1r_d[:, :])
            nc.sync.dma_start(out=w2pf[:, :], in_=w2p_d[:, :])
            nc.sync.dma_start(out=w2rf[:, :], in_=w2r_d[:, :])
            nc.sync.dma_start(out=b0s[:, :], in_=b0_d[:, :])
            nc.sync.dma_start(out=b1s[:, :], in_=b1_d[:, :])
            nc.sync.dma_start(out=b2s[:, :], in_=b2_d[:, :])
            nc.vector.tensor_copy(out=w0b[:, :], in_=w0f[:, :])
            nc.vector.tensor_copy(out=w1pb[:, :], in_=w1pf[:, :])
            nc.vector.tensor_copy(out=w1rb[:, :], in_=w1rf[:, :])
            nc.vector.tensor_copy(out=w2pb[:, :], in_=w2pf[:, :])
            nc.vector.tensor_copy(out=w2rb[:, :], in_=w2rf[:, :])

            # x slab, loaded in 4 row pieces so conv0 can start early.
            # SWDGE (gpsimd) + 4KB descriptors spreads across the SDMA fleet.
            xb = bigp.tile([CIN, XR, W], BF16, tag="big")
            for r0, r1 in ((0, 36), (36, 72), (72, 104), (104, XR)):
                nc.gpsimd.dma_start(out=xb[:, r0:r1, :], in_=x_d[:, r0:r1, :],
                                    max_dma_last_dim=2048)

            c1s = c1p.tile([2 * CF, C1R, C1W], BF16, tag="c1s")
            # conv0: 3x3 valid, 103 -> 64. Two row-chunks run concurrently
            # in the two PE column halves (col-tiling via tile_position).
            for yo in range(0, C1R, 4):
                ps = psp.tile([2 * CF, 2, C1W], F32, tag="ps")
                k = 0
                for dy in range(3):
                    for dx in range(3):
                        t = dy * 3 + dx
                        for h in range(2):
                            nc.tensor.matmul(
                                out=ps[h * CF:(h + 1) * CF, :, :],
                                lhsT=w0b[:, t * CF:(t + 1) * CF],
                                rhs=xb[:, yo + 2 * h + dy:yo + 2 * h + dy + 2,
                                       dx:dx + C1W],
                                start=(k == 0), stop=(k == 8),
                                tile_position=(0, h * CF),
                            )
                        k += 1
                for h in range(2):
                    yh = yo + 2 * h
                    nc.scalar.activation(
                        out=c1s[0:CF, yh:yh + 2, :],
                        in_=ps[h * CF:(h + 1) * CF, :, :],
                        func=RELU, bias=b0s[:, 0:1],
                    )
                    if yh >= 2:
                        nc.vector.tensor_copy(
                            out=c1s[CF:2 * CF, yh - 2:yh, :],
                            in_=c1s[0:CF, yh:yh + 2, :],
                        )
            for r0, r1 in ((0, 34), (34, 68), (68, 102), (102, C1R)):
                nc.sync.dma_start(out=c1_d[:, r0:r1, :], in_=c1s[0:CF, r0:r1, :])

            c2s = bigp.tile([CF, C2R, C2W], BF16, tag="big")
            pool = poolp.tile([2 * CF, PR, PW], BF16, tag="pool")
            # conv1: 3x3 dilation 2, valid, 64 -> 64; dy0+dy1 packed into
            # a 128-deep contraction, two row-chunks col-tiled. The 2x2 pool
            # (sans 0.25, folded into w2) is interleaved per chunk.
            for yo in range(0, C2R, 4):
                ps = psp.tile([2 * CF, 2, C2W], F32, tag="ps")
                for dx in range(3):
                    for h in range(2):
                        nc.tensor.matmul(
                            out=ps[h * CF:(h + 1) * CF, :, :],
                            lhsT=w1pb[:, dx * CF:(dx + 1) * CF],
                            rhs=c1s[0:2 * CF, yo + 2 * h:yo + 2 * h + 2,
                                    2 * dx:2 * dx + C2W],
                            start=(dx == 0), stop=False,
                            tile_position=(0, h * CF),
                        )
                for dx in range(3):
                    for h in range(2):
                        nc.tensor.matmul(
                            out=ps[h * CF:(h + 1) * CF, :, :],
                            lhsT=w1rb[:, dx * CF:(dx + 1) * CF],
                            rhs=c1s[0:CF, yo + 2 * h + 4:yo + 2 * h + 6,
                                    2 * dx:2 * dx + C2W],
                            start=False, stop=(dx == 2),
                            tile_position=(0, h * CF),
                        )
                for h in range(2):
                    yh = yo + 2 * h
                    nc.scalar.activation(
                        out=c2s[:, yh:yh + 2, :],
                        in_=ps[h * CF:(h + 1) * CF, :, :],
                        func=RELU, bias=b1s[:, 0:1],
                    )
                # pool rows p0, p0+1 from c2 rows yo..yo+3
                p0 = yo // 2
                rsum = rsp.tile([CF, 2, C2W], BF16, tag="rs")
                nc.vector.tensor_add(
                    out=rsum[:, :, :],
                    in0=c2s[:, yo:yo + 4:2, :], in1=c2s[:, yo + 1:yo + 4:2, :])
                nc.vector.tensor_add(
                    out=pool[0:CF, p0:p0 + 2, :],
                    in0=rsum[:, :, 0:2 * PW:2], in1=rsum[:, :, 1:2 * PW:2])
                # packed pool partitions: pool[64+ch, l] = pool[ch, l+3]
                if p0 == 4:
                    nc.vector.tensor_copy(
                        out=pool[CF:2 * CF, 0:3, :], in_=pool[0:CF, 3:6, :])
                elif p0 > 4:
                    nc.vector.tensor_copy(
                        out=pool[CF:2 * CF, p0 - 3:p0 - 1, :],
                        in_=pool[0:CF, p0:p0 + 2, :])
            for r0, r1 in ((0, 33), (33, 66), (66, 99), (99, C2R)):
                nc.sync.dma_start(out=c2_d[:, r0:r1, :], in_=c2s[:, r0:r1, :])

            c3s = c3p.tile([CF, C3R, C3W], BF16, tag="c3s")
            # conv2: 3x3 dilation 3, valid, 64 -> 64; dy0+dy1 packed,
            # two 4-row chunks col-tiled.
            for so in range(0, C3R, 8):
                ns = [min(4, C3R - so), min(4, max(0, C3R - so - 4))]
                nhalf = 2 if ns[1] > 0 else 1
                ps = psp.tile([2 * CF, 4, C3W], F32, tag="ps")
                for dx in range(3):
                    for h in range(nhalf):
                        nc.tensor.matmul(
                            out=ps[h * CF:(h + 1) * CF, 0:ns[h], :],
                            lhsT=w2pb[:, dx * CF:(dx + 1) * CF],
                            rhs=pool[0:2 * CF, so + 4 * h:so + 4 * h + ns[h],
                                     3 * dx:3 * dx + C3W],
                            start=(dx == 0), stop=False,
                            tile_position=(0, h * CF),
                        )
                for dx in range(3):
                    for h in range(nhalf):
                        nc.tensor.matmul(
                            out=ps[h * CF:(h + 1) * CF, 0:ns[h], :],
                            lhsT=w2rb[:, dx * CF:(dx + 1) * CF],
                            rhs=pool[0:CF, so + 4 * h + 6:so + 4 * h + 6 + ns[h],
                                     3 * dx:3 * dx + C3W],
                            start=False, stop=(dx == 2),
                            tile_position=(0, h * CF),
                        )
                for h in range(nhalf):
                    sh = so + 4 * h
                    nc.scalar.activation(
                        out=c3s[:, sh:sh + ns[h], :],
                        in_=ps[h * CF:(h + 1) * CF, 0:ns[h], :],
                        func=RELU, bias=b2s[:, 0:1],
                    )
            nc.sync.dma_start(out=c3_d[:, :, :], in_=c3s[:, :, :])
    nc.compile()
    return nc


def _get_compiled():
    if "nc" not in _CACHE:
        _CACHE["nc"] = _build(
            bacc.Bacc("TRN2", target_bir_lowering=False, debug=False, num_devices=8)
        )
    return _CACHE["nc"]


def _upsample(x, Ho, Wo):
    """bilinear, align_corners=True, float32"""
    def mat1(n_out, n_in):
        idx = np.arange(n_out, dtype=np.float64) * ((n_in - 1) / (n_out - 1))
        lo = np.clip(np.floor(idx).astype(np.int64), 0, n_in - 2)
        f = (idx - lo).astype(np.float32)
        return lo, f
    lo, f = mat1(Ho, x.shape[2])
    x = x[:, :, lo, :] * (1.0 - f)[None, None, :, None] \
        + x[:, :, lo + 1, :] * f[None, None, :, None]
    lo, f = mat1(Wo, x.shape[3])
    x = x[:, :, :, lo] * (1.0 - f)[None, None, None, :] \
        + x[:, :, :, lo + 1] * f[None, None, None, :]
    return x.astype(np.float32)


def kernel(x, conv0_w, conv0_b, conv1_w, conv1_b, conv2_w, conv2_b, convp_w,
           fc1_w, fc2_w, enc_w, codewords, scale, attn_w, attn_b,
           bn_w, bn_b, cls_w, cls_b):
    global LAST_RESULT
    x = np.asarray(x, np.float32)
    to32 = lambda a: np.asarray(a, np.float32)
    (conv0_w, conv0_b, conv1_w, conv1_b, conv2_w, conv2_b, convp_w, fc1_w,
     fc2_w, enc_w, codewords, scale, attn_w, attn_b, bn_w, bn_b, cls_w,
     cls_b) = map(to32, (conv0_w, conv0_b, conv1_w, conv1_b, conv2_w, conv2_b,
                         convp_w, fc1_w, fc2_w, enc_w, codewords, scale,
                         attn_w, attn_b, bn_w, bn_b, cls_w, cls_b))

    nc = _get_compiled()

    w0t = np.ascontiguousarray(
        conv0_w.transpose(1, 2, 3, 0).reshape(CIN, 9 * CF))
    w1t_full = conv1_w.transpose(1, 2, 3, 0).reshape(CF, 3, 3, CF)
    w1p = np.zeros((2 * CF, 3 * CF), np.float32)
    for dx in range(3):
        w1p[:CF, dx * CF:(dx + 1) * CF] = w1t_full[:, 0, dx, :]
        w1p[CF:, dx * CF:(dx + 1) * CF] = w1t_full[:, 1, dx, :]
    w1r = np.ascontiguousarray(w1t_full[:, 2, :, :].reshape(CF, 3 * CF))
    # conv2 weights absorb the 2x2-avgpool's 0.25 (device pool only sums)
    w2t_full = 0.25 * conv2_w.transpose(1, 2, 3, 0).reshape(CF, 3, 3, CF)
    w2p = np.zeros((2 * CF, 3 * CF), np.float32)
    for dx in range(3):
        w2p[:CF, dx * CF:(dx + 1) * CF] = w2t_full[:, 0, dx, :]
        w2p[CF:, dx * CF:(dx + 1) * CF] = w2t_full[:, 1, dx, :]
    w2r = np.ascontiguousarray(w2t_full[:, 2, :, :].reshape(CF, 3 * CF))
    common = {
        "w0t": w0t, "w1p": w1p, "w1r": w1r, "w2p": w2p, "w2r": w2r,
        "b0": conv0_b.reshape(CF, 1).copy(),
        "b1": conv1_b.reshape(CF, 1).copy(),
        "b2": conv2_b.reshape(CF, 1).copy(),
    }
    in_maps = []
    for i in range(8):
        b, half = i // 2, i % 2
        xs = np.zeros((CIN, XR, W), np.float32)
        if half == 0:
            xs[:, :, :] = x[b, :, 0:XR, :]
        else:
            xs[:, 0:H - 120, :] = x[b, :, 120:H, :]
        in_maps.append({"x": xs.astype(ml_dtypes.bfloat16), **common})

    res = run_bass_kernel_spmd(nc, in_maps, core_ids=list(range(8)))
    LAST_RESULT = res

    c1 = np.empty((B, CF, 254, 254), np.float32)
    c2 = np.empty((B, CF, 250, 250), np.float32)
    c3 = np.empty((B, CF, 119, 119), np.float32)
    for i in range(8):
        b, half = i // 2, i % 2
        r = res.results[i]
        s1 = np.asarray(r["c1"]).astype(np.float32)
        s2 = np.asarray(r["c2"]).astype(np.float32)
        s3 = np.asarray(r["c3"]).astype(np.float32)
        if half == 0:
            c1[b, :, 0:127, :] = s1[:, 0:127, :]
            c2[b, :, 0:125, :] = s2[:, 0:125, :]
            c3[b, :, 0:60, :] = s3[:, 0:60, :]
        else:
            c1[b, :, 127:254, :] = s1[:, 7:134, :]
            c2[b, :, 125:250, :] = s2[:, 5:130, :]
            c3[b, :, 60:119, :] = s3[:, 0:59, :]

    # ---- host: CBAM channel attention ----
    sig = lambda v: 1.0 / (1.0 + np.exp(-v))

    def fc(v):
        return np.maximum(v @ fc1_w.T, 0.0) @ fc2_w.T

    gate_c = sig(fc(c3.mean((2, 3))) + fc(c3.max((2, 3))))
    xg = gate_c[:, :, None, None] * c3
    # ---- spatial attention: 7x7 conv, pad 3, 2 -> 1 channel ----
    sp = np.stack([xg.mean(1), xg.max(1)], 1)          # (B,2,119,119)
    pad = convp_w.shape[-1] // 2
    spp = np.pad(sp, ((0, 0), (0, 0), (pad, pad), (pad, pad)))
    conv_sp = np.zeros((B, C3W, C3W), np.float32)
    for c in range(2):
        for dy in range(7):
            for dx in range(7):
                conv_sp += convp_w[0, c, dy, dx] * \
                    spp[:, c, dy:dy + C3W, dx:dx + C3W]
    x_p = sig(conv_sp)[:, None, :, :] * c3
    xx = c3 + x_p
    # ---- soft-VQ context encoding (batch folded into n, as in reference) ----
    K, D = codewords.shape
    feat = np.maximum(np.tensordot(enc_w, xx, axes=([1], [1])), 0.0)
    feat = feat.transpose(1, 0, 2, 3)                  # (B, D, h, w)
    Zf = feat.reshape(1, D, -1)[0].T                   # (n, D)
    norm = ((Zf ** 2).sum(-1, keepdims=True)
            + (codewords ** 2).sum(-1)[None, :]
            - 2.0 * Zf @ codewords.T)
    logit = scale[None, :] * norm
    logit -= logit.max(1, keepdims=True)
    Aexp = np.exp(logit)
    A = Aexp / Aexp.sum(1, keepdims=True)              # (n, K)
    E = A.T @ Zf - A.sum(0)[:, None] * codewords       # (K, D)
    mu = E.mean(1, keepdims=True)
    var = ((E - mu) ** 2).mean(1, keepdims=True)
    En = (E - mu) / np.sqrt(var + 1e-5) * bn_w[:, None] + bn_b[:, None]
    E_sum = np.maximum(En, 0.0).sum(0)[None, :]        # (1, D)
    gamma = sig(E_sum @ attn_w.T + attn_b).reshape(-1, CF, 1, 1)
    xx = xx + xx * gamma
    # ---- upsample + concat + classifier ----
    cat = np.concatenate(
        [_upsample(c1, H, W), _upsample(c2, H, W), _upsample(xx, H, W)], 1)
    out = np.tensordot(cls_w, cat, axes=([1], [1])).transpose(1, 0, 2, 3)
    out = out + cls_b[None, :, None, None]
    return np.ascontiguousarray(out.astype(np.float32))


# revision 12
# speedup vs baseline: 3.4247x; 1.7115x over previous
"""Distributed Trainium2 kernel for nn_CAnet (vq_codebook).

Sharding: (batch, H-half) -> 8 cores. Core i handles batch i//2, H-half i%2.
Halos are materialized host-side by overlapping the input row slabs, so the
device graph needs no collectives. The device computes the dominant stages
(conv0 3x3, conv1 3x3 d2, 2x2 avgpool, conv2 3x3 d3, all with ReLU) as
PSUM-accumulated per-tap matmuls in bf16. The cheap global stages (CBAM
attention, soft-VQ encoding, bilinear upsample, classifier) run host-side.

The avgpool's 0.25 scale is folded into the conv2 weights host-side, so the
device pool stage is two adds (row-add contiguous, col-add strided).
"""

import numpy as np
import ml_dtypes

from concourse import bacc, mybir, tile
from concourse.bass_utils import run_bass_kernel_spmd

F32 = mybir.dt.float32
BF16 = mybir.dt.bfloat16

B = 4
CIN = 103
CP = 128    # channels padded to the full partition grid (zeros 103..127) so
            # DMA descriptor->engine assignment uses the clean 16x8 swizzle
CF = 64
H = W = 256
# per-core geometry: one batch, one H-half (+halos, zero-padded at the edge)
XR = 138    # x rows per shard (half0: 0..138, half1: 120..256 padded to 138)
C1R = 136   # c1 rows computed per shard
C2R = 132   # c2 rows per shard
PR = 66     # pooled rows per shard
C3R = 60    # c3 rows per shard
C1W, C2W, PW, C3W = 254, 250, 125, 119

_CACHE = {}
LAST_RESULT = None


def _build(nc):
    x_d = nc.dram_tensor("x", [CP, XR, W], BF16, kind="ExternalInput").ap()
    # all conv weights pre-transposed + bf16, packed into one [128, 1920]:
    # cols 0:576 conv0 taps, 576:768 w1p, 768:960 w1r, 960:1152 w2p, 1152:1344 w2r
    wall_d = nc.dram_tensor("wall", [CP, 1344], BF16, kind="ExternalInput").ap()
    bs_d = nc.dram_tensor("bs", [CF, 3], F32, kind="ExternalInput").ap()
    c1_d = nc.dram_tensor("c1", [CF, C1R, C1W], BF16, kind="ExternalOutput").ap()
    c2_d = nc.dram_tensor("c2", [CF, C2R, C2W], BF16, kind="ExternalOutput").ap()
    c3_d = nc.dram_tensor("c3", [CF, C3R, C3W], BF16, kind="ExternalOutput").ap()

    RELU = mybir.ActivationFunctionType.Relu

    with tile.TileContext(nc) as tc:
        with (
            tc.tile_pool(name="wpool", bufs=1) as wpool,
            tc.tile_pool(name="big", bufs=1) as bigp,
            tc.tile_pool(name="c1p", bufs=1) as c1p,
            tc.tile_pool(name="poolp", bufs=1) as poolp,
            tc.tile_pool(name="c3p", bufs=1) as c3p,
            tc.tile_pool(name="rs", bufs=2) as rsp,
            tc.tile_pool(name="ps", bufs=6, space="PSUM") as psp,
        ):
            wall = wpool.tile([CP, 1344], BF16, tag="wall")
            bs = wpool.tile([CF, 3], F32, tag="bs")
            nc.sync.dma_start(out=wall[:, :], in_=wall_d[:, :])
            nc.sync.dma_start(out=bs[:, :], in_=bs_d[:, :])

            # x slab, loaded in 4 row pieces so conv0 can start early.
            xb = bigp.tile([CP, XR, W], BF16, tag="big")
            for r0, r1 in ((0, 36), (36, 72), (72, 104), (104, XR)):
                nc.gpsimd.dma_start(out=xb[:, r0:r1, :], in_=x_d[:, r0:r1, :])

            c1s = c1p.tile([2 * CF, C1R, C1W], BF16, tag="c1s")
            # conv0: 3x3 valid, 103 -> 64. Two row-chunks run concurrently
            # in the two PE column halves (col-tiling via tile_position).
            for yo in range(0, C1R, 4):
                ps = psp.tile([2 * CF, 2, C1W], F32, tag="ps")
                k = 0
                for dy in range(3):
                    for dx in range(3):
                        t = dy * 3 + dx
                        for h in range(2):
                            nc.tensor.matmul(
                                out=ps[h * CF:(h + 1) * CF, :, :],
                                lhsT=wall[:, t * CF:(t + 1) * CF],
                                rhs=xb[:, yo + 2 * h + dy:yo + 2 * h + dy + 2,
                                       dx:dx + C1W],
                                start=(k == 0), stop=(k == 8),
                                tile_position=(0, h * CF),
                            )
                        k += 1
                for h in range(2):
                    yh = yo + 2 * h
                    nc.scalar.activation(
                        out=c1s[0:CF, yh:yh + 2, :],
                        in_=ps[h * CF:(h + 1) * CF, :, :],
                        func=RELU, bias=bs[:, 0:1],
                    )
                    if yh >= 2:
                        nc.vector.tensor_copy(
                            out=c1s[CF:2 * CF, yh - 2:yh, :],
                            in_=c1s[0:CF, yh:yh + 2, :],
                        )
            for r0, r1 in ((0, 34), (34, 68), (68, 102), (102, C1R)):
                nc.sync.dma_start(out=c1_d[:, r0:r1, :], in_=c1s[0:CF, r0:r1, :])

            c2s = bigp.tile([CF, C2R, C2W], BF16, tag="big")
            pool = poolp.tile([2 * CF, PR, PW], BF16, tag="pool")
            # conv1: 3x3 dilation 2, valid, 64 -> 64; dy0+dy1 packed into
            # a 128-deep contraction, two row-chunks col-tiled. The 2x2 pool
            # (sans 0.25, folded into w2) is interleaved per chunk.
            for yo in range(0, C2R, 4):
                ps = psp.tile([2 * CF, 2, C2W], F32, tag="ps")
                for dx in range(3):
                    for h in range(2):
                        nc.tensor.matmul(
                            out=ps[h * CF:(h + 1) * CF, :, :],
                            lhsT=wall[:, 576 + dx * CF:576 + (dx + 1) * CF],
                            rhs=c1s[0:2 * CF, yo + 2 * h:yo + 2 * h + 2,
                                    2 * dx:2 * dx + C2W],
                            start=(dx == 0), stop=False,
                            tile_position=(0, h * CF),
                        )
                for dx in range(3):
                    for h in range(2):
                        nc.tensor.matmul(
                            out=ps[h * CF:(h + 1) * CF, :, :],
                            lhsT=wall[0:CF, 768 + dx * CF:768 + (dx + 1) * CF],
                            rhs=c1s[0:CF, yo + 2 * h + 4:yo + 2 * h + 6,
                                    2 * dx:2 * dx + C2W],
                            start=False, stop=(dx == 2),
                            tile_position=(0, h * CF),
                        )
                for h in range(2):
                    yh = yo + 2 * h
                    nc.scalar.activation(
                        out=c2s[:, yh:yh + 2, :],
                        in_=ps[h * CF:(h + 1) * CF, :, :],
                        func=RELU, bias=bs[:, 1:2],
                    )
                # pool rows p0, p0+1 from c2 rows yo..yo+3
                p0 = yo // 2
                rsum = rsp.tile([CF, 2, C2W], BF16, tag="rs")
                nc.vector.tensor_add(
                    out=rsum[:, :, :],
                    in0=c2s[:, yo:yo + 4:2, :], in1=c2s[:, yo + 1:yo + 4:2, :])
                nc.vector.tensor_add(
                    out=pool[0:CF, p0:p0 + 2, :],
                    in0=rsum[:, :, 0:2 * PW:2], in1=rsum[:, :, 1:2 * PW:2])
                # packed pool partitions: pool[64+ch, l] = pool[ch, l+3]
                if p0 == 4:
                    nc.vector.tensor_copy(
                        out=pool[CF:2 * CF, 0:3, :], in_=pool[0:CF, 3:6, :])
                elif p0 > 4:
                    nc.vector.tensor_copy(
                        out=pool[CF:2 * CF, p0 - 3:p0 - 1, :],
                        in_=pool[0:CF, p0:p0 + 2, :])
            for r0, r1 in ((0, 33), (33, 66), (66, 99), (99, C2R)):
                nc.sync.dma_start(out=c2_d[:, r0:r1, :], in_=c2s[:, r0:r1, :])

            c3s = c3p.tile([CF, C3R, C3W], BF16, tag="c3s")
            # conv2: 3x3 dilation 3, valid, 64 -> 64; dy0+dy1 packed,
            # two 4-row chunks col-tiled.
            for so in range(0, C3R, 8):
                ns = [min(4, C3R - so), min(4, max(0, C3R - so - 4))]
                nhalf = 2 if ns[1] > 0 else 1
                ps = psp.tile([2 * CF, 4, C3W], F32, tag="ps")
                for dx in range(3):
                    for h in range(nhalf):
                        nc.tensor.matmul(
                            out=ps[h * CF:(h + 1) * CF, 0:ns[h], :],
                            lhsT=wall[:, 960 + dx * CF:960 + (dx + 1) * CF],
                            rhs=pool[0:2 * CF, so + 4 * h:so + 4 * h + ns[h],
                                     3 * dx:3 * dx + C3W],
                            start=(dx == 0), stop=False,
                            tile_position=(0, h * CF),
                        )
                for dx in range(3):
                    for h in range(nhalf):
                        nc.tensor.matmul(
                            out=ps[h * CF:(h + 1) * CF, 0:ns[h], :],
                            lhsT=wall[0:CF, 1152 + dx * CF:1152 + (dx + 1) * CF],
                            rhs=pool[0:CF, so + 4 * h + 6:so + 4 * h + 6 + ns[h],
                                     3 * dx:3 * dx + C3W],
                            start=False, stop=(dx == 2),
                            tile_position=(0, h * CF),
                        )
                for h in range(nhalf):
                    sh = so + 4 * h
                    nc.scalar.activation(
                        out=c3s[:, sh:sh + ns[h], :],
                        in_=ps[h * CF:(h + 1) * CF, 0:ns[h], :],
                        func=RELU, bias=bs[:, 2:3],
                    )
            nc.sync.dma_start(out=c3_d[:, :, :], in_=c3s[:, :, :])
    nc.compile()
    return nc


def _get_compiled():
    if "nc" not in _CACHE:
        _CACHE["nc"] = _build(
            bacc.Bacc("TRN2", target_bir_lowering=False, debug=False, num_devices=8)
        )
    return _CACHE["nc"]


def _upsample(x, Ho, Wo):
    """bilinear, align_corners=True, float32"""
    def mat1(n_out, n_in):
        idx = np.arange(n_out, dtype=np.float64) * ((n_in - 1) / (n_out - 1))
        lo = np.clip(np.floor(idx).astype(np.int64), 0, n_in - 2)
        f = (idx - lo).astype(np.float32)
        return lo, f
    lo, f = mat1(Ho, x.shape[2])
    x = x[:, :, lo, :] * (1.0 - f)[None, None, :, None] \
        + x[:, :, lo + 1, :] * f[None, None, :, None]
    lo, f = mat1(Wo, x.shape[3])
    x = x[:, :, :, lo] * (1.0 - f)[None, None, None, :] \
        + x[:, :, :, lo + 1] * f[None, None, None, :]
    return x.astype(np.float32)


def kernel(x, conv0_w, conv0_b, conv1_w, conv1_b, conv2_w, conv2_b, convp_w,
           fc1_w, fc2_w, enc_w, codewords, scale, attn_w, attn_b,
           bn_w, bn_b, cls_w, cls_b):
    global LAST_RESULT
    x = np.asarray(x, np.float32)
    to32 = lambda a: np.asarray(a, np.float32)
    (conv0_w, conv0_b, conv1_w, conv1_b, conv2_w, conv2_b, convp_w, fc1_w,
     fc2_w, enc_w, codewords, scale, attn_w, attn_b, bn_w, bn_b, cls_w,
     cls_b) = map(to32, (conv0_w, conv0_b, conv1_w, conv1_b, conv2_w, conv2_b,
                         convp_w, fc1_w, fc2_w, enc_w, codewords, scale,
                         attn_w, attn_b, bn_w, bn_b, cls_w, cls_b))

    nc = _get_compiled()

    wall = np.zeros((CP, 1344), np.float32)
    wall[:CIN, 0:576] = conv0_w.transpose(1, 2, 3, 0).reshape(CIN, 9 * CF)
    w1t_full = conv1_w.transpose(1, 2, 3, 0).reshape(CF, 3, 3, CF)
    for dx in range(3):
        wall[:CF, 576 + dx * CF:576 + (dx + 1) * CF] = w1t_full[:, 0, dx, :]
        wall[CF:2 * CF, 576 + dx * CF:576 + (dx + 1) * CF] = w1t_full[:, 1, dx, :]
    wall[:CF, 768:960] = w1t_full[:, 2, :, :].reshape(CF, 3 * CF)
    # conv2 weights absorb the 2x2-avgpool's 0.25 (device pool only sums)
    w2t_full = 0.25 * conv2_w.transpose(1, 2, 3, 0).reshape(CF, 3, 3, CF)
    for dx in range(3):
        wall[:CF, 960 + dx * CF:960 + (dx + 1) * CF] = w2t_full[:, 0, dx, :]
        wall[CF:2 * CF, 960 + dx * CF:960 + (dx + 1) * CF] = w2t_full[:, 1, dx, :]
    wall[:CF, 1152:1344] = w2t_full[:, 2, :, :].reshape(CF, 3 * CF)
    bsv = np.stack([conv0_b, conv1_b, conv2_b], 1).astype(np.float32)
    common = {"wall": wall.astype(ml_dtypes.bfloat16), "bs": bsv.copy()}
    in_maps = []
    for i in range(8):
        b, half = i // 2, i % 2
        xs = np.zeros((CP, XR, W), np.float32)
        if half == 0:
            xs[:CIN, :, :] = x[b, :, 0:XR, :]
        else:
            xs[:CIN, 0:H - 120, :] = x[b, :, 120:H, :]
        in_maps.append({"x": xs.astype(ml_dtypes.bfloat16), **common})

    res = run_bass_kernel_spmd(nc, in_maps, core_ids=list(range(8)))
    LAST_RESULT = res

    c1 = np.empty((B, CF, 254, 254), np.float32)
    c2 = np.empty((B, CF, 250, 250), np.float32)
    c3 = np.empty((B, CF, 119, 119), np.float32)
    for i in range(8):
        b, half = i // 2, i % 2
        r = res.results[i]
        s1 = np.asarray(r["c1"]).astype(np.float32)
        s2 = np.asarray(r["c2"]).astype(np.float32)
        s3 = np.asarray(r["c3"]).astype(np.float32)
        if half == 0:
            c1[b, :, 0:127, :] = s1[:, 0:127, :]
            c2[b, :, 0:125, :] = s2[:, 0:125, :]
            c3[b, :, 0:60, :] = s3[:, 0:60, :]
        else:
            c1[b, :, 127:254, :] = s1[:, 7:134, :]
            c2[b, :, 125:250, :] = s2[:, 5:130, :]
            c3[b, :, 60:119, :] = s3[:, 0:59, :]

    # ---- host: CBAM channel attention ----
    sig = lambda v: 1.0 / (1.0 + np.exp(-v))

    def fc(v):
        return np.maximum(v @ fc1_w.T, 0.0) @ fc2_w.T

    gate_c = sig(fc(c3.mean((2, 3))) + fc(c3.max((2, 3))))
    xg = gate_c[:, :, None, None] * c3
    # ---- spatial attention: 7x7 conv, pad 3, 2 -> 1 channel ----
    sp = np.stack([xg.mean(1), xg.max(1)], 1)          # (B,2,119,119)
    pad = convp_w.shape[-1] // 2
    spp = np.pad(sp, ((0, 0), (0, 0), (pad, pad), (pad, pad)))
    conv_sp = np.zeros((B, C3W, C3W), np.float32)
    for c in range(2):
        for dy in range(7):
            for dx in range(7):
                conv_sp += convp_w[0, c, dy, dx] * \
                    spp[:, c, dy:dy + C3W, dx:dx + C3W]
    x_p = sig(conv_sp)[:, None, :, :] * c3
    xx = c3 + x_p
    # ---- soft-VQ context encoding (batch folded into n, as in reference) ----
    K, D = codewords.shape
    feat = np.maximum(np.tensordot(enc_w, xx, axes=([1], [1])), 0.0)
    feat = feat.transpose(1, 0, 2, 3)                  # (B, D, h, w)
    Zf = feat.reshape(1, D, -1)[0].T                   # (n, D)
    norm = ((Zf ** 2).sum(-1, keepdims=True)
            + (codewords ** 2).sum(-1)[None, :]
            - 2.0 * Zf @ codewords.T)
    logit = scale[None, :] * norm
    logit -= logit.max(1, keepdims=True)
    Aexp = np.exp(logit)
    A = Aexp / Aexp.sum(1, keepdims=True)              # (n, K)
    E = A.T @ Zf - A.sum(0)[:, None] * codewords       # (K, D)
    mu = E.mean(1, keepdims=True)
    var = ((E - mu) ** 2).mean(1, keepdims=True)
    En = (E - mu) / np.sqrt(var + 1e-5) * bn_w[:, None] + bn_b[:, None]
    E_sum = np.maximum(En, 0.0).sum(0)[None, :]        # (1, D)
    gamma = sig(E_sum @ attn_w.T + attn_b).reshape(-1, CF, 1, 1)
    xx = xx + xx * gamma
    # ---- upsample + concat + classifier ----
    cat = np.concatenate(
        [_upsample(c1, H, W), _upsample(c2, H, W), _upsample(xx, H, W)], 1)
    out = np.tensordot(cls_w, cat, axes=([1], [1])).transpose(1, 0, 2, 3)
    out = out + cls_b[None, :, None, None]
    return np.ascontiguousarray(out.astype(np.float32))


# revision 14
# speedup vs baseline: 3.6655x; 1.0703x over previous
"""Distributed Trainium2 kernel for nn_CAnet (vq_codebook).

Sharding: (batch, H-half) -> 8 cores. Core i handles batch i//2, H-half i%2.
Halos are materialized host-side by overlapping the input row slabs, so the
device graph needs no collectives. The device computes the dominant stages
(conv0 3x3, conv1 3x3 d2, 2x2 avgpool, conv2 3x3 d3, all with ReLU) as
PSUM-accumulated per-tap matmuls in bf16. The cheap global stages (CBAM
attention, soft-VQ encoding, bilinear upsample, classifier) run host-side.

The avgpool's 0.25 scale is folded into the conv2 weights host-side, so the
device pool stage is two adds (row-add contiguous, col-add strided).
"""

import numpy as np
import ml_dtypes

from concourse import bacc, mybir, tile
from concourse.bass_utils import run_bass_kernel_spmd

F32 = mybir.dt.float32
BF16 = mybir.dt.bfloat16

B = 4
CIN = 103
CP = 128    # channels padded to the full partition grid (zeros 103..127) so
            # DMA descriptor->engine assignment uses the clean 16x8 swizzle
CF = 64
H = W = 256
# per-core geometry: one batch, one H-half (+halos, zero-padded at the edge)
XR = 138    # x rows per shard (half0: 0..138, half1: 120..256 padded to 138)
C1R = 136   # c1 rows computed per shard
C2R = 132   # c2 rows per shard
PR = 66     # pooled rows per shard
C3R = 60    # c3 rows per shard
C1W, C2W, PW, C3W = 254, 250, 125, 119

_CACHE = {}
LAST_RESULT = None


def _build(nc):
    x_d = nc.dram_tensor("x", [CP, XR, W], BF16, kind="ExternalInput").ap()
    # all conv weights pre-transposed + bf16, packed into one [128, 1920]:
    # cols 0:576 conv0 taps, 576:768 w1p, 768:960 w1r, 960:1152 w2p, 1152:1344 w2r
    wall_d = nc.dram_tensor("wall", [CP, 1344], BF16, kind="ExternalInput").ap()
    bs_d = nc.dram_tensor("bs", [CF, 3], F32, kind="ExternalInput").ap()
    c1_d = nc.dram_tensor("c1", [CF, C1R, C1W], BF16, kind="ExternalOutput").ap()
    c2_d = nc.dram_tensor("c2", [CF, C2R, C2W], BF16, kind="ExternalOutput").ap()
    c3_d = nc.dram_tensor("c3", [CF, C3R, C3W], BF16, kind="ExternalOutput").ap()

    RELU = mybir.ActivationFunctionType.Relu

    with tile.TileContext(nc) as tc:
        with (
            tc.tile_pool(name="wpool", bufs=1) as wpool,
            tc.tile_pool(name="big", bufs=1) as bigp,
            tc.tile_pool(name="c1p", bufs=1) as c1p,
            tc.tile_pool(name="poolp", bufs=1) as poolp,
            tc.tile_pool(name="c3p", bufs=1) as c3p,
            tc.tile_pool(name="rs", bufs=2) as rsp,
            tc.tile_pool(name="ps", bufs=6, space="PSUM") as psp,
        ):
            wall = wpool.tile([CP, 1344], BF16, tag="wall")
            bs = wpool.tile([CF, 3], F32, tag="bs")
            # everything on the SWDGE (gpsimd) queue: Q7 descriptor gen is
            # ~35ns/desc and the data spreads across all 16 SDMA engines for
            # clean 128-partition transfers. Small first x piece so conv0
            # starts ~10us in.
            nc.gpsimd.dma_start(out=wall[:, :], in_=wall_d[:, :])
            nc.gpsimd.dma_start(out=bs[:, :], in_=bs_d[:, :])
            xb = bigp.tile([CP, XR, W], BF16, tag="big")
            for r0, r1 in ((0, 12), (12, 36), (36, 70), (70, 104), (104, XR)):
                nc.gpsimd.dma_start(out=xb[:, r0:r1, :], in_=x_d[:, r0:r1, :])

            c1s = c1p.tile([2 * CF, C1R, C1W], BF16, tag="c1s")
            # conv0: 3x3 valid, 103 -> 64. Two row-chunks run concurrently
            # in the two PE column halves (col-tiling via tile_position).
            for yo in range(0, C1R, 4):
                ps = psp.tile([2 * CF, 2, C1W], F32, tag="ps")
                k = 0
                for dy in range(3):
                    for dx in range(3):
                        t = dy * 3 + dx
                        for h in range(2):
                            nc.tensor.matmul(
                                out=ps[h * CF:(h + 1) * CF, :, :],
                                lhsT=wall[:, t * CF:(t + 1) * CF],
                                rhs=xb[:, yo + 2 * h + dy:yo + 2 * h + dy + 2,
                                       dx:dx + C1W],
                                start=(k == 0), stop=(k == 8),
                                tile_position=(0, h * CF),
                            )
                        k += 1
                for h in range(2):
                    yh = yo + 2 * h
                    nc.scalar.activation(
                        out=c1s[0:CF, yh:yh + 2, :],
                        in_=ps[h * CF:(h + 1) * CF, :, :],
                        func=RELU, bias=bs[:, 0:1],
                    )
                    if yh >= 2:
                        nc.vector.tensor_copy(
                            out=c1s[CF:2 * CF, yh - 2:yh, :],
                            in_=c1s[0:CF, yh:yh + 2, :],
                        )
            for r0, r1 in ((0, 34), (34, 68), (68, 102), (102, C1R)):
                nc.sync.dma_start(out=c1_d[:, r0:r1, :], in_=c1s[0:CF, r0:r1, :])

            c2s = bigp.tile([CF, C2R, C2W], BF16, tag="big")
            pool = poolp.tile([2 * CF, PR, PW], BF16, tag="pool")
            # conv1: 3x3 dilation 2, valid, 64 -> 64; dy0+dy1 packed into
            # a 128-deep contraction, two row-chunks col-tiled. The 2x2 pool
            # (sans 0.25, folded into w2) is interleaved per chunk.
            for yo in range(0, C2R, 4):
                ps = psp.tile([2 * CF, 2, C2W], F32, tag="ps")
                for dx in range(3):
                    for h in range(2):
                        nc.tensor.matmul(
                            out=ps[h * CF:(h + 1) * CF, :, :],
                            lhsT=wall[:, 576 + dx * CF:576 + (dx + 1) * CF],
                            rhs=c1s[0:2 * CF, yo + 2 * h:yo + 2 * h + 2,
                                    2 * dx:2 * dx + C2W],
                            start=(dx == 0), stop=False,
                            tile_position=(0, h * CF),
                        )
                for dx in range(3):
                    for h in range(2):
                        nc.tensor.matmul(
                            out=ps[h * CF:(h + 1) * CF, :, :],
                            lhsT=wall[0:CF, 768 + dx * CF:768 + (dx + 1) * CF],
                            rhs=c1s[0:CF, yo + 2 * h + 4:yo + 2 * h + 6,
                                    2 * dx:2 * dx + C2W],
                            start=False, stop=(dx == 2),
                            tile_position=(0, h * CF),
                        )
                for h in range(2):
                    yh = yo + 2 * h
                    nc.scalar.activation(
                        out=c2s[:, yh:yh + 2, :],
                        in_=ps[h * CF:(h + 1) * CF, :, :],
                        func=RELU, bias=bs[:, 1:2],
                    )
                # pool rows p0, p0+1 from c2 rows yo..yo+3
                p0 = yo // 2
                rsum = rsp.tile([CF, 2, C2W], BF16, tag="rs")
                nc.vector.tensor_add(
                    out=rsum[:, :, :],
                    in0=c2s[:, yo:yo + 4:2, :], in1=c2s[:, yo + 1:yo + 4:2, :])
                nc.vector.tensor_add(
                    out=pool[0:CF, p0:p0 + 2, :],
                    in0=rsum[:, :, 0:2 * PW:2], in1=rsum[:, :, 1:2 * PW:2])
                # packed pool partitions: pool[64+ch, l] = pool[ch, l+3]
                if p0 == 4:
                    nc.vector.tensor_copy(
                        out=pool[CF:2 * CF, 0:3, :], in_=pool[0:CF, 3:6, :])
                elif p0 > 4:
                    nc.vector.tensor_copy(
                        out=pool[CF:2 * CF, p0 - 3:p0 - 1, :],
                        in_=pool[0:CF, p0:p0 + 2, :])
            for r0, r1 in ((0, 33), (33, 66), (66, 99), (99, C2R)):
                nc.sync.dma_start(out=c2_d[:, r0:r1, :], in_=c2s[:, r0:r1, :])

            c3s = c3p.tile([CF, C3R, C3W], BF16, tag="c3s")
            # conv2: 3x3 dilation 3, valid, 64 -> 64; dy0+dy1 packed,
            # two 4-row chunks col-tiled.
            for so in range(0, C3R, 8):
                ns = [min(4, C3R - so), min(4, max(0, C3R - so - 4))]
                nhalf = 2 if ns[1] > 0 else 1
                ps = psp.tile([2 * CF, 4, C3W], F32, tag="ps")
                for dx in range(3):
                    for h in range(nhalf):
                        nc.tensor.matmul(
                            out=ps[h * CF:(h + 1) * CF, 0:ns[h], :],
                            lhsT=wall[:, 960 + dx * CF:960 + (dx + 1) * CF],
                            rhs=pool[0:2 * CF, so + 4 * h:so + 4 * h + ns[h],
                                     3 * dx:3 * dx + C3W],
                            start=(dx == 0), stop=False,
                            tile_position=(0, h * CF),
                        )
                for dx in range(3):
                    for h in range(nhalf):
                        nc.tensor.matmul(
                            out=ps[h * CF:(h + 1) * CF, 0:ns[h], :],
                            lhsT=wall[0:CF, 1152 + dx * CF:1152 + (dx + 1) * CF],
                            rhs=pool[0:CF, so + 4 * h + 6:so + 4 * h + 6 + ns[h],
                                     3 * dx:3 * dx + C3W],
                            start=False, stop=(dx == 2),
                            tile_position=(0, h * CF),
                        )
                for h in range(nhalf):
                    sh = so + 4 * h
                    nc.scalar.activation(
                        out=c3s[:, sh:sh + ns[h], :],
                        in_=ps[h * CF:(h + 1) * CF, 0:ns[h], :],
                        func=RELU, bias=bs[:, 2:3],
                    )
                if so == 24:
                    nc.sync.dma_start(out=c3_d[:, 0:32, :], in_=c3s[:, 0:32, :])
            nc.sync.dma_start(out=c3_d[:, 32:C3R, :], in_=c3s[:, 32:C3R, :])
    nc.compile()
    return nc


def _get_compiled():
    if "nc" not in _CACHE:
        _CACHE["nc"] = _build(
            bacc.Bacc("TRN2", target_bir_lowering=False, debug=False, num_devices=8)
        )
    return _CACHE["nc"]


def _upsample(x, Ho, Wo):
    """bilinear, align_corners=True, float32"""
    def mat1(n_out, n_in):
        idx = np.arange(n_out, dtype=np.float64) * ((n_in - 1) / (n_out - 1))
        lo = np.clip(np.floor(idx).astype(np.int64), 0, n_in - 2)
        f = (idx - lo).astype(np.float32)
        return lo, f
    lo, f = mat1(Ho, x.shape[2])
    x = x[:, :, lo, :] * (1.0 - f)[None, None, :, None] \
        + x[:, :, lo + 1, :] * f[None, None, :, None]
    lo, f = mat1(Wo, x.shape[3])
    x = x[:, :, :, lo] * (1.0 - f)[None, None, None, :] \
        + x[:, :, :, lo + 1] * f[None, None, None, :]
    return x.astype(np.float32)


def kernel(x, conv0_w, conv0_b, conv1_w, conv1_b, conv2_w, conv2_b, convp_w,
           fc1_w, fc2_w, enc_w, codewords, scale, attn_w, attn_b,
           bn_w, bn_b, cls_w, cls_b):
    global LAST_RESULT
    x = np.asarray(x, np.float32)
    to32 = lambda a: np.asarray(a, np.float32)
    (conv0_w, conv0_b, conv1_w, conv1_b, conv2_w, conv2_b, convp_w, fc1_w,
     fc2_w, enc_w, codewords, scale, attn_w, attn_b, bn_w, bn_b, cls_w,
     cls_b) = map(to32, (conv0_w, conv0_b, conv1_w, conv1_b, conv2_w, conv2_b,
                         convp_w, fc1_w, fc2_w, enc_w, codewords, scale,
                         attn_w, attn_b, bn_w, bn_b, cls_w, cls_b))

    nc = _get_compiled()

    wall = np.zeros((CP, 1344), np.float32)
    wall[:CIN, 0:576] = conv0_w.transpose(1, 2, 3, 0).reshape(CIN, 9 * CF)
    w1t_full = conv1_w.transpose(1, 2, 3, 0).reshape(CF, 3, 3, CF)
    for dx in range(3):
        wall[:CF, 576 + dx * CF:576 + (dx + 1) * CF] = w1t_full[:, 0, dx, :]
        wall[CF:2 * CF, 576 + dx * CF:576 + (dx + 1) * CF] = w1t_full[:, 1, dx, :]
    wall[:CF, 768:960] = w1t_full[:, 2, :, :].reshape(CF, 3 * CF)
    # conv2 weights absorb the 2x2-avgpool's 0.25 (device pool only sums)
    w2t_full = 0.25 * conv2_w.transpose(1, 2, 3, 0).reshape(CF, 3, 3, CF)
    for dx in range(3):
        wall[:CF, 960 + dx * CF:960 + (dx + 1) * CF] = w2t_full[:, 0, dx, :]
        wall[CF:2 * CF, 960 + dx * CF:960 + (dx + 1) * CF] = w2t_full[:, 1, dx, :]
    wall[:CF, 1152:1344] = w2t_full[:, 2, :, :].reshape(CF, 3 * CF)
    bsv = np.stack([conv0_b, conv1_b, conv2_b], 1).astype(np.float32)
    common = {"wall": wall.astype(ml_dtypes.bfloat16), "bs": bsv.copy()}
    in_maps = []
    for i in range(8):
        b, half = i // 2, i % 2
        xs = np.zeros((CP, XR, W), np.float32)
        if half == 0:
            xs[:CIN, :, :] = x[b, :, 0:XR, :]
        else:
            xs[:CIN, 0:H - 120, :] = x[b, :, 120:H, :]
        in_maps.append({"x": xs.astype(ml_dtypes.bfloat16), **common})

    res = run_bass_kernel_spmd(nc, in_maps, core_ids=list(range(8)))
    LAST_RESULT = res

    c1 = np.empty((B, CF, 254, 254), np.float32)
    c2 = np.empty((B, CF, 250, 250), np.float32)
    c3 = np.empty((B, CF, 119, 119), np.float32)
    for i in range(8):
        b, half = i // 2, i % 2
        r = res.results[i]
        s1 = np.asarray(r["c1"]).astype(np.float32)
        s2 = np.asarray(r["c2"]).astype(np.float32)
        s3 = np.asarray(r["c3"]).astype(np.float32)
        if half == 0:
            c1[b, :, 0:127, :] = s1[:, 0:127, :]
            c2[b, :, 0:125, :] = s2[:, 0:125, :]
            c3[b, :, 0:60, :] = s3[:, 0:60, :]
        else:
            c1[b, :, 127:254, :] = s1[:, 7:134, :]
            c2[b, :, 125:250, :] = s2[:, 5:130, :]
            c3[b, :, 60:119, :] = s3[:, 0:59, :]

    # ---- host: CBAM channel attention ----
    sig = lambda v: 1.0 / (1.0 + np.exp(-v))

    def fc(v):
        return np.maximum(v @ fc1_w.T, 0.0) @ fc2_w.T

    gate_c = sig(fc(c3.mean((2, 3))) + fc(c3.max((2, 3))))
    xg = gate_c[:, :, None, None] * c3
    # ---- spatial attention: 7x7 conv, pad 3, 2 -> 1 channel ----
    sp = np.stack([xg.mean(1), xg.max(1)], 1)          # (B,2,119,119)
    pad = convp_w.shape[-1] // 2
    spp = np.pad(sp, ((0, 0), (0, 0), (pad, pad), (pad, pad)))
    conv_sp = np.zeros((B, C3W, C3W), np.float32)
    for c in range(2):
        for dy in range(7):
            for dx in range(7):
                conv_sp += convp_w[0, c, dy, dx] * \
                    spp[:, c, dy:dy + C3W, dx:dx + C3W]
    x_p = sig(conv_sp)[:, None, :, :] * c3
    xx = c3 + x_p
    # ---- soft-VQ context encoding (batch folded into n, as in reference) ----
    K, D = codewords.shape
    feat = np.maximum(np.tensordot(enc_w, xx, axes=([1], [1])), 0.0)
    feat = feat.transpose(1, 0, 2, 3)                  # (B, D, h, w)
    Zf = feat.reshape(1, D, -1)[0].T                   # (n, D)
    norm = ((Zf ** 2).sum(-1, keepdims=True)
            + (codewords ** 2).sum(-1)[None, :]
            - 2.0 * Zf @ codewords.T)
    logit = scale[None, :] * norm
    logit -= logit.max(1, keepdims=True)
    Aexp = np.exp(logit)
    A = Aexp / Aexp.sum(1, keepdims=True)              # (n, K)
    E = A.T @ Zf - A.sum(0)[:, None] * codewords       # (K, D)
    mu = E.mean(1, keepdims=True)
    var = ((E - mu) ** 2).mean(1, keepdims=True)
    En = (E - mu) / np.sqrt(var + 1e-5) * bn_w[:, None] + bn_b[:, None]
    E_sum = np.maximum(En, 0.0).sum(0)[None, :]        # (1, D)
    gamma = sig(E_sum @ attn_w.T + attn_b).reshape(-1, CF, 1, 1)
    xx = xx + xx * gamma
    # ---- upsample + concat + classifier ----
    cat = np.concatenate(
        [_upsample(c1, H, W), _upsample(c2, H, W), _upsample(xx, H, W)], 1)
    out = np.tensordot(cls_w, cat, axes=([1], [1])).transpose(1, 0, 2, 3)
    out = out + cls_b[None, :, None, None]
    return np.ascontiguousarray(out.astype(np.float32))
